# revision 3
# baseline (speedup 1.0000x reference)
"""Trainium2 Bass kernel for nn_Actions_Emb (ragged char-mean + action embedding).

Algorithm (per core, data-parallel over batch):
  out[b, 0, :]   = action_table[98]                       (BOS broadcast)
  out[b, 1+s, :] = slot_type==0 ? mean(char_table[ids])   (masked by char_len)
                 : slot_type==1 ? action_table[action_id]
                 : 0

Device-side formulation: for each slot build a weight vector
  W_char[c] = count(c in ids[:len]) * (type==0)/len        (58 classes)
  W_act[a]  = (a == action_id_masked)                      (99 classes)
then   out_slot = W_char @ char_table + W_act @ action_table
via two accumulating PE matmuls per 128-slot tile. Counts are built on DVE
with iota-compare accumulate (per-partition scalar = char id), transposed to
class-major with the PE, and fed to the matmuls.
"""

import numpy as np
import sys

if "/opt/trn_rl_repo" not in sys.path:
    sys.path.insert(0, "/opt/trn_rl_repo")

import concourse.bass as bass
import concourse.bacc as bacc
import concourse.mybir as mybir
import concourse.tile as tile
from concourse.bass_utils import run_bass_kernel_spmd
from concourse.masks import make_identity

B, S, L, D = 16384, 4, 16, 256
NCHAR, NACT, BOS_ID = 58, 99, 98
NCORES = 8
B_CORE = B // NCORES           # 2048 proof steps per core
SLOTS = B_CORE * S             # 8192 slots per core
P = 128
NT = SLOTS // P                # 64 tiles of 128 slots
TPP = NT                       # slots per partition (slot = p*NT + t)

f32 = mybir.dt.float32
i32 = mybir.dt.int32
Alu = mybir.AluOpType

_CACHE = {}


def build_nc():
    nc = bacc.Bacc("TRN2", target_bir_lowering=False, debug=False,
                   num_devices=NCORES)

    ids_d = nc.dram_tensor("char_ids", [SLOTS, L], i32, kind="ExternalInput")
    len_d = nc.dram_tensor("char_len", [SLOTS], i32, kind="ExternalInput")
    act_d = nc.dram_tensor("action_ids", [SLOTS], i32, kind="ExternalInput")
    typ_d = nc.dram_tensor("slot_type", [SLOTS], i32, kind="ExternalInput")
    ct_d = nc.dram_tensor("char_table", [NCHAR, D], f32, kind="ExternalInput")
    at_d = nc.dram_tensor("action_table", [NACT, D], f32, kind="ExternalInput")
    out_d = nc.dram_tensor("out", [B_CORE * 5, D], f32, kind="ExternalOutput")

    # slot (local) = p*NT + t ; proof step b = slot//4 = p*16 + t//4 ; j = t%4
    # output row = b*5 + 1 + j = 80*p + 5*(t//4) + 1 + (t%4)
    ids_r = ids_d.rearrange("(p t) l -> p t l", p=P)        # [128, 64, 16]
    len_r = len_d.rearrange("(p t) -> p t", p=P)            # [128, 64]
    act_r = act_d.rearrange("(p t) -> p t", p=P)
    typ_r = typ_d.rearrange("(p t) -> p t", p=P)
    out_r = out_d.rearrange("(p x) d -> p x d", p=P)        # [128, 80, 256]

    from contextlib import ExitStack
    with tile.TileContext(nc) as tc, ExitStack() as es:
        consts = es.enter_context(tc.tile_pool(name="consts", bufs=1))
        big = es.enter_context(tc.tile_pool(name="big", bufs=1))

        # ---- constants ----
        ident = consts.tile([P, P], f32)
        make_identity(nc, ident)

        iota58_i = consts.tile([P, NCHAR], i32)
        nc.gpsimd.iota(iota58_i, pattern=[[1, NCHAR]], base=0,
                       channel_multiplier=0)
        iota58 = consts.tile([P, NCHAR], f32)
        nc.vector.tensor_copy(iota58, iota58_i)

        iota99_i = consts.tile([P, NACT], i32)
        nc.gpsimd.iota(iota99_i, pattern=[[1, NACT]], base=0,
                       channel_multiplier=0)
        iota99 = consts.tile([P, NACT], f32)
        nc.vector.tensor_copy(iota99, iota99_i)

        iota16_i = consts.tile([P, L], i32)
        nc.gpsimd.iota(iota16_i, pattern=[[1, L]], base=0,
                       channel_multiplier=0)
        iota16 = consts.tile([P, L], f32)
        nc.vector.tensor_copy(iota16, iota16_i)

        ct_sb = consts.tile([NCHAR, D], f32)
        nc.sync.dma_start(ct_sb, ct_d[:, :])
        at_sb = consts.tile([NACT, D], f32)
        nc.sync.dma_start(at_sb, at_d[:, :])

        # BOS row broadcast to all partitions
        bos1 = consts.tile([1, D], f32)
        nc.sync.dma_start(bos1, at_d[BOS_ID:BOS_ID + 1, :])
        bos_sb = consts.tile([P, D], f32)
        nc.gpsimd.partition_broadcast(bos_sb, bos1)

        # ---- bulk input loads ----
        ids_i = big.tile([P, NT, L], i32)
        nc.sync.dma_start(ids_i, ids_r)
        len_i = big.tile([P, NT], i32)
        nc.sync.dma_start(len_i, len_r)
        act_i = big.tile([P, NT], i32)
        nc.sync.dma_start(act_i, act_r)
        typ_i = big.tile([P, NT], i32)
        nc.sync.dma_start(typ_i, typ_r)

        # ---- hoisted scalar prep ----
        lenf = big.tile([P, NT], f32)
        nc.vector.tensor_copy(lenf, len_i)
        rlen = big.tile([P, NT], f32)
        nc.vector.reciprocal(rlen, lenf)
        t0 = big.tile([P, NT], f32)
        nc.vector.tensor_scalar(out=t0, in0=typ_i, scalar1=0.0, scalar2=None,
                                op0=Alu.is_equal)
        s0 = big.tile([P, NT], f32)
        nc.vector.tensor_tensor(out=s0, in0=t0, in1=rlen, op=Alu.mult)

        # action id with sentinel for non-action slots: act + 128*(type!=1)
        u = big.tile([P, NT], f32)
        nc.vector.tensor_scalar(out=u, in0=typ_i, scalar1=1.0, scalar2=None,
                                op0=Alu.is_equal)
        nc.vector.tensor_scalar(out=u, in0=u, scalar1=-128.0, scalar2=128.0,
                                op0=Alu.mult, op1=Alu.add)
        act_m = big.tile([P, NT], f32)
        nc.vector.tensor_tensor(out=act_m, in0=act_i, in1=u, op=Alu.add)

        # masked char ids: ids + 64*(l >= len)  (sentinel never matches 0..57)
        m = big.tile([P, NT, L], f32)
        for t in range(NT):
            nc.vector.tensor_scalar(out=m[:, t], in0=iota16,
                                    scalar1=lenf[:, t:t + 1], scalar2=None,
                                    op0=Alu.is_ge)
        nc.vector.tensor_scalar(out=m, in0=m, scalar1=64.0, scalar2=None,
                                op0=Alu.mult)
        ids_m = big.tile([P, NT, L], f32)
        nc.vector.tensor_tensor(out=ids_m, in0=ids_i, in1=m, op=Alu.add)

        # ---- BOS output ----
        for k in range(16):
            nc.sync.dma_start(out_r[:, 5 * k, :], bos_sb)

        # ---- main slot pipeline ----
        with (
            tc.tile_pool(name="w", bufs=3) as wpool,
            tc.tile_pool(name="tp", bufs=2, space="PSUM") as tpp,
            tc.tile_pool(name="op", bufs=2, space="PSUM") as opp,
            tc.tile_pool(name="ob", bufs=3) as obuf,
        ):
            for t in range(NT):
                # counts: W[p, c] = sum_l (ids_m[p,t,l] == c)
                w = wpool.tile([P, NCHAR], f32, tag="w")
                nc.vector.tensor_scalar(out=w, in0=iota58,
                                        scalar1=ids_m[:, t, 0:1], scalar2=None,
                                        op0=Alu.is_equal)
                for l in range(1, L):
                    nc.vector.scalar_tensor_tensor(
                        out=w, in0=iota58, scalar=ids_m[:, t, l:l + 1], in1=w,
                        op0=Alu.is_equal, op1=Alu.add)
                # scale by (type==0)/len
                wc = wpool.tile([P, NCHAR], f32, tag="wc")
                nc.vector.tensor_scalar(out=wc, in0=w, scalar1=s0[:, t:t + 1],
                                        scalar2=None, op0=Alu.mult)
                # action one-hot (sentinel-masked)
                wa = wpool.tile([P, NACT], f32, tag="wa")
                nc.vector.tensor_scalar(out=wa, in0=iota99,
                                        scalar1=act_m[:, t:t + 1], scalar2=None,
                                        op0=Alu.is_equal)

                # transpose to class-major
                wct_p = tpp.tile([NCHAR, P], f32, tag="wct")
                nc.tensor.transpose(wct_p, wc, ident)
                wat_p = tpp.tile([NACT, P], f32, tag="wat")
                nc.tensor.transpose(wat_p, wa, ident)
                wct = wpool.tile([NCHAR, P], f32, tag="wct_s")
                nc.scalar.copy(wct, wct_p)
                wat = wpool.tile([NACT, P], f32, tag="wat_s")
                nc.scalar.copy(wat, wat_p)

                # out = Wc.T @ char_table + Wa.T @ action_table
                out_p = opp.tile([P, D], f32, tag="out_p")
                nc.tensor.matmul(out_p, wct, ct_sb, start=True, stop=False)
                nc.tensor.matmul(out_p, wat, at_sb, start=False, stop=True)
                out_sb = obuf.tile([P, D], f32, tag="out_sb")
                nc.scalar.copy(out_sb, out_p)

                x = 5 * (t // 4) + 1 + (t % 4)
                nc.sync.dma_start(out_r[:, x, :], out_sb)

    nc.compile()
    return nc


def kernel(**inputs):
    char_ids = np.ascontiguousarray(np.asarray(inputs["char_ids"], np.int32))
    char_len = np.ascontiguousarray(np.asarray(inputs["char_len"], np.int32))
    action_ids = np.ascontiguousarray(np.asarray(inputs["action_ids"], np.int32))
    slot_type = np.ascontiguousarray(np.asarray(inputs["slot_type"], np.int32))
    char_table = np.ascontiguousarray(np.asarray(inputs["char_table"], np.float32))
    action_table = np.ascontiguousarray(np.asarray(inputs["action_table"], np.float32))

    ids_f = char_ids.reshape(B * S, L)
    len_f = char_len.reshape(B * S)
    act_f = action_ids.reshape(B * S)
    typ_f = slot_type.reshape(B * S)

    if "nc" not in _CACHE:
        _CACHE["nc"] = build_nc()
    nc = _CACHE["nc"]

    in_maps = []
    for c in range(NCORES):
        sl = slice(c * SLOTS, (c + 1) * SLOTS)
        # reorder rows so that slot_local = p*NT + t lands at [p, t]
        in_maps.append({
            "char_ids": ids_f[sl],
            "char_len": len_f[sl],
            "action_ids": act_f[sl],
            "slot_type": typ_f[sl],
            "char_table": char_table,
            "action_table": action_table,
        })

    res = run_bass_kernel_spmd(nc, in_maps, list(range(NCORES)))
    out = np.empty((B, 5, D), np.float32)
    for c in range(NCORES):
        out[c * B_CORE:(c + 1) * B_CORE] = (
            res.results[c]["out"].reshape(B_CORE, 5, D))
    return out


if __name__ == "__main__":
    import reference
    inp = {k: np.asarray(v) for k, v in reference.setup_inputs().items()}
    got = kernel(**inp)
    exp = np.asarray(reference.reference(**inp))
    err = np.abs(got - exp).max() / (np.abs(exp).max() + 1e-9)
    print("rel err:", err)
